# revision 1
# baseline (speedup 1.0000x reference)
"""LoRA linear kernel for Trainium2 (8 NeuronCores, SPMD data-parallel).

Computes y = x @ (B @ A)^T for
    x: [4, 2048, 4096] f32, B: [4096, 16] f32, A: [16, 4096] f32.

Strategy: never materialize W = B @ A.  Factor as t = x @ A^T (rank 16)
then y = t @ B^T.  Tokens (4*2048 = 8192) are sharded across 8 cores
(1024 tokens each); A and B are replicated.  The host pre-transposes x
into feature-major layout ([128, 32, TOK] = f-on-partitions) so every
device DMA is a perfectly contiguous pattern and the tensor engine can
contract over features directly.

Matmuls run in float32r (fp32 with 11-bit mantissa, single-pass PE mode,
4x the rate of 2-pass LOW_HIGH fp32).  Inputs are round-to-nearest
pre-rounded on the host so the truncation is unbiased.

Per-core dataflow:
  mm1: t^T[16, tok]  = sum_ko  A^T[ko]  (lhsT [128,16]) . x^T[ko] (rhs [128,tok])
  mm2: y[tok128, o]  = t^T[:, chunk] (lhsT [16,128])    . B^T     (rhs [16,512])
  y DMA'd out in natural token-major layout -> host just concatenates.
"""

import sys

import numpy as np

if "/opt/trn_rl_repo" not in sys.path:
    sys.path.insert(0, "/opt/trn_rl_repo")

# Problem shape (hardcoded per contract)
BATCH = 4
SEQ = 2048
D = 4096          # in_features == out_features
R = 16            # lora rank
NCORES = 8
NTOK = BATCH * SEQ            # 8192 tokens total
TOK = NTOK // NCORES          # 1024 tokens per core
P = 128                       # partitions
KO = D // P                   # 32 feature chunks
TB = 512                      # token block for mm1
NB = 512                      # matmul free dim for mm2 (fp32 max)
XC = 8                        # feature chunks per x DMA (2MB pieces)

# Module-level knobs for test.py (harness never touches these)
TRACE = False
LAST_RESULTS = None

_nc_cache = None


def _round_f32r(a):
    """Round fp32 array to f32r (11-bit mantissa) with round-to-nearest-even."""
    v = np.ascontiguousarray(a, dtype=np.float32).view(np.uint32)
    lsb = (v >> np.uint32(12)) & np.uint32(1)
    r = (v + np.uint32(0x7FF) + lsb) & np.uint32(0xFFFFF000)
    return r.view(np.float32)


def _build_program():
    from concourse import bacc, mybir, tile

    # Bacc (not raw Bass): its finalize() runs generate_event_semaphores,
    # which splits multi-sem waits to satisfy TRN2's 1-wait-per-instruction
    # hardware constraint (walrus rejects >1 otherwise).
    nc = bacc.Bacc(
        "TRN2", target_bir_lowering=False, debug=False, num_devices=NCORES
    )

    f32 = mybir.dt.float32
    f32r = mybir.dt.float32r

    xt = nc.dram_tensor("xt", [P, KO, TOK], f32r, kind="ExternalInput")
    at = nc.dram_tensor("at", [P, KO, R], f32r, kind="ExternalInput")
    bt = nc.dram_tensor("bt", [R, D], f32r, kind="ExternalInput")
    y = nc.dram_tensor("y", [TOK, D], f32, kind="ExternalOutput")

    with tile.TileContext(nc) as tc:
        with (
            tc.tile_pool(name="consts", bufs=1) as consts,
            tc.tile_pool(name="xin", bufs=12) as xin,
            tc.tile_pool(name="tbuf", bufs=2) as tbuf,
            tc.tile_pool(name="yout", bufs=2) as yout,
            tc.tile_pool(name="pt", bufs=2, space="PSUM") as pt_pool,
            tc.tile_pool(name="py", bufs=6, space="PSUM") as py_pool,
        ):
            at_s = consts.tile([P, KO, R], f32r)
            nc.sync.dma_start(at_s[:], at[:])
            bt_s = consts.tile([R, D], f32r)
            nc.sync.dma_start(bt_s[:], bt[:])

            # Warm-up matmuls: (a) make PE observe the at/bt DMA sems early,
            # (b) keep PE streaming during the x-DMA prologue so the HAM
            # clock gate reaches K=8/8 before the real matmuls start.
            obs1 = py_pool.tile([R, R], f32, tag="psum_y")
            nc.tensor.matmul(obs1[:], at_s[:, 0, :], at_s[:, 0, :R], start=True, stop=True)
            for _ in range(6):
                warm = py_pool.tile([P, NB], f32, tag="psum_y")
                nc.tensor.matmul(warm[:], bt_s[:, :P], bt_s[:, :NB], start=True, stop=True)
            tc.no_sync_barrier()

            # ko-chunks per x DMA: small first pieces so mm1 starts early
            XCS = [2, 2, 4, 4, 4, 4, 4, 4, 4]
            n_blocks = TOK // TB
            assert n_blocks == 2

            def load_x(tb):
                xts = []  # list of (tile, ko_base, width)
                ko_base = 0
                for w in XCS:
                    xt_tile = xin.tile([P, w, TB], f32r, tag="xt")
                    nc.sync.dma_start(
                        xt_tile[:],
                        xt[:, ko_base : ko_base + w, tb * TB : (tb + 1) * TB],
                    )
                    xts.append((xt_tile, ko_base, w))
                    ko_base += w
                return xts

            def mm1_range(xts, psum_t, ko_lo, ko_hi):
                for xt_tile, kb, w in xts:
                    for j in range(w):
                        ko = kb + j
                        if ko_lo <= ko < ko_hi:
                            nc.tensor.matmul(
                                psum_t[:],
                                at_s[:, ko, :],
                                xt_tile[:, j, :],
                                start=(ko == 0),
                                stop=(ko == KO - 1),
                            )

            def round_t(psum_t):
                # DVE copy fp32 -> f32r: the rounding step the verifier wants
                tT = tbuf.tile([R, TB], f32r)
                nc.vector.tensor_copy(tT[:], psum_t[:])
                return tT

            def mm2_chunk(tb, c, tT):
                y_row = yout.tile([P, D], f32)
                for n in range(D // NB):
                    psum_y = py_pool.tile([P, NB], f32, tag="psum_y")
                    nc.tensor.matmul(
                        psum_y[:],
                        tT[:, c * P : (c + 1) * P],
                        bt_s[:, n * NB : (n + 1) * NB],
                        start=True,
                        stop=True,
                    )
                    # Alternate PSUM-evacuation between DVE and ACT so
                    # neither engine gates the tensor engine's psum slots
                    if n % 3 == 2:
                        nc.scalar.copy(y_row[:, n * NB : (n + 1) * NB], psum_y[:])
                    else:
                        nc.vector.tensor_copy(y_row[:, n * NB : (n + 1) * NB], psum_y[:])
                row0 = tb * TB + c * P
                # scalar-engine HWDGE ring: offloads the Sync sequencer
                nc.scalar.dma_start(y[row0 : row0 + P, :], y_row[:])

            # PE order must follow x-arrival order (PE is FIFO: a matmul
            # waiting on a late DMA blocks everything behind it).
            for tb in range(n_blocks):
                xts = load_x(tb)
                psum_t = pt_pool.tile([R, TB], f32, tag="psum_t")
                mm1_range(xts, psum_t, 0, KO)
                tT = round_t(psum_t)
                for c in range(TB // P):
                    mm2_chunk(tb, c, tT)

    nc.finalize()
    return nc


def kernel(x, lora_matrix_B, lora_matrix_A):
    global _nc_cache, LAST_RESULTS
    from concourse.bass_utils import run_bass_kernel_spmd

    if _nc_cache is None:
        _nc_cache = _build_program()
    nc = _nc_cache

    x_flat = _round_f32r(np.asarray(x, dtype=np.float32)).reshape(NTOK, D)
    A = _round_f32r(np.asarray(lora_matrix_A, dtype=np.float32))
    B = _round_f32r(np.asarray(lora_matrix_B, dtype=np.float32))

    # at[p, ko, j] = A[j, ko*128 + p];  bt[j, o] = B[o, j]
    at_prep = np.ascontiguousarray(A.reshape(R, KO, P).transpose(2, 1, 0))
    bt_prep = np.ascontiguousarray(B.T)

    in_maps = []
    for c in range(NCORES):
        xc = x_flat[c * TOK : (c + 1) * TOK, :]
        # xt[p, ko, t] = xc[t, ko*128 + p]
        xt_prep = np.ascontiguousarray(xc.reshape(TOK, KO, P).transpose(2, 1, 0))
        in_maps.append({"xt": xt_prep, "at": at_prep, "bt": bt_prep})

    res = run_bass_kernel_spmd(
        nc, in_maps, core_ids=list(range(NCORES)), trace=TRACE
    )
    LAST_RESULTS = res

    y = np.concatenate([res.results[c]["y"] for c in range(NCORES)], axis=0)
    return y.reshape(BATCH, SEQ, D)



# revision 2
# speedup vs baseline: 1.6106x; 1.6106x over previous
"""LoRA linear kernel for Trainium2 (8 NeuronCores, SPMD data-parallel).

Computes y = x @ (B @ A)^T for
    x: [4, 2048, 4096] f32, B: [4096, 16] f32, A: [16, 4096] f32.

Strategy: never materialize W = B @ A.  Factor as t = x @ A^T (rank 16)
then y = t @ B^T.  Tokens (4*2048 = 8192) are sharded across 8 cores
(1024 tokens each); A and B are replicated.  The host pre-transposes x
into feature-major layout ([128, 32, TOK] = f-on-partitions) so every
device DMA is a perfectly contiguous pattern and the tensor engine can
contract over features directly.

The kernel is HBM-bandwidth bound (~358 GB/s/core), so x is staged and
y is returned in float16 (tolerance is 2e-2; fp16 end-to-end gives
~7e-4), halving HBM traffic vs fp32.  Matmuls run fp16 x fp16 with
fp32 PSUM accumulation.

Per-core dataflow:
  mm1: t^T[16, tok]  = sum_ko  A^T[ko]  (lhsT [128,16]) . x^T[ko] (rhs [128,tok])
  mm2: y[tok128, o]  = t^T[:, chunk] (lhsT [16,128])    . B^T     (rhs [16,512])
  y DMA'd out in natural token-major layout -> host just concatenates.
"""

import sys

import numpy as np

if "/opt/trn_rl_repo" not in sys.path:
    sys.path.insert(0, "/opt/trn_rl_repo")

# Problem shape (hardcoded per contract)
BATCH = 4
SEQ = 2048
D = 4096          # in_features == out_features
R = 16            # lora rank
NCORES = 8
NTOK = BATCH * SEQ            # 8192 tokens total
TOK = NTOK // NCORES          # 1024 tokens per core
P = 128                       # partitions
KO = D // P                   # 32 feature chunks
TB = 512                      # token block for mm1
NB = 512                      # matmul free dim for mm2 (psum bank limit)

# Module-level knobs for test.py (harness never touches these)
TRACE = False
LAST_RESULTS = None

_nc_cache = None


def _build_program():
    from concourse import bacc, mybir, tile

    # Bacc (not raw Bass): its finalize() runs generate_event_semaphores,
    # which splits multi-sem waits to satisfy TRN2's 1-wait-per-instruction
    # hardware constraint (walrus rejects >1 otherwise).
    nc = bacc.Bacc(
        "TRN2", target_bir_lowering=False, debug=False, num_devices=NCORES
    )

    f32 = mybir.dt.float32
    f16 = mybir.dt.float16

    xt = nc.dram_tensor("xt", [P, KO, TOK], f16, kind="ExternalInput")
    at = nc.dram_tensor("at", [P, KO, R], f16, kind="ExternalInput")
    bt = nc.dram_tensor("bt", [R, D], f16, kind="ExternalInput")
    y = nc.dram_tensor("y", [TOK, D], f16, kind="ExternalOutput")

    with tile.TileContext(nc) as tc:
        with (
            tc.tile_pool(name="consts", bufs=1) as consts,
            tc.tile_pool(name="xin", bufs=12) as xin,
            tc.tile_pool(name="tbuf", bufs=2) as tbuf,
            tc.tile_pool(name="yout", bufs=2) as yout,
            tc.tile_pool(name="pt", bufs=2, space="PSUM") as pt_pool,
            tc.tile_pool(name="py", bufs=6, space="PSUM") as py_pool,
        ):
            at_s = consts.tile([P, KO, R], f16)
            nc.sync.dma_start(at_s[:], at[:])
            bt_s = consts.tile([R, D], f16)
            nc.sync.dma_start(bt_s[:], bt[:])

            # Warm-up matmuls: (a) make PE observe the at/bt DMA sems early,
            # (b) keep PE streaming during the x-DMA prologue so the HAM
            # clock gate reaches K=8/8 before the real matmuls start.
            obs1 = py_pool.tile([R, R], f32, tag="psum_y")
            nc.tensor.matmul(obs1[:], at_s[:, 0, :], at_s[:, 0, :R], start=True, stop=True)
            for _ in range(6):
                warm = py_pool.tile([P, NB], f32, tag="psum_y")
                nc.tensor.matmul(warm[:], bt_s[:, :P], bt_s[:, :NB], start=True, stop=True)
            tc.no_sync_barrier()

            # ko-chunks per x DMA: small first pieces so mm1 starts early
            XCS = [2, 2, 4, 4, 4, 4, 4, 4, 4]
            n_blocks = TOK // TB
            assert n_blocks == 2

            def load_x(tb):
                xts = []  # list of (tile, ko_base, width)
                ko_base = 0
                for w in XCS:
                    xt_tile = xin.tile([P, w, TB], f16, tag="xt")
                    nc.sync.dma_start(
                        xt_tile[:],
                        xt[:, ko_base : ko_base + w, tb * TB : (tb + 1) * TB],
                    )
                    xts.append((xt_tile, ko_base, w))
                    ko_base += w
                return xts

            def mm1_range(xts, psum_t, ko_lo, ko_hi):
                for xt_tile, kb, w in xts:
                    for j in range(w):
                        ko = kb + j
                        if ko_lo <= ko < ko_hi:
                            nc.tensor.matmul(
                                psum_t[:],
                                at_s[:, ko, :],
                                xt_tile[:, j, :],
                                start=(ko == 0),
                                stop=(ko == KO - 1),
                            )

            def round_t(psum_t):
                # DVE copy fp32 -> f16 for the mm2 stationary operand
                tT = tbuf.tile([R, TB], f16)
                nc.vector.tensor_copy(tT[:], psum_t[:])
                return tT

            def mm2_chunk(tb, c, tT):
                y_row = yout.tile([P, D], f16)
                for n in range(D // NB):
                    psum_y = py_pool.tile([P, NB], f32, tag="psum_y")
                    nc.tensor.matmul(
                        psum_y[:],
                        tT[:, c * P : (c + 1) * P],
                        bt_s[:, n * NB : (n + 1) * NB],
                        start=True,
                        stop=True,
                    )
                    # Alternate PSUM-evacuation between DVE and ACT so
                    # neither engine gates the tensor engine's psum slots
                    if n % 3 == 2:
                        nc.scalar.copy(y_row[:, n * NB : (n + 1) * NB], psum_y[:])
                    else:
                        nc.vector.tensor_copy(y_row[:, n * NB : (n + 1) * NB], psum_y[:])
                row0 = tb * TB + c * P
                # scalar-engine HWDGE ring: offloads the Sync sequencer
                nc.scalar.dma_start(y[row0 : row0 + P, :], y_row[:])

            # PE order must follow x-arrival order (PE is FIFO: a matmul
            # waiting on a late DMA blocks everything behind it).
            for tb in range(n_blocks):
                xts = load_x(tb)
                psum_t = pt_pool.tile([R, TB], f32, tag="psum_t")
                mm1_range(xts, psum_t, 0, KO)
                tT = round_t(psum_t)
                for c in range(TB // P):
                    mm2_chunk(tb, c, tT)

    nc.finalize()
    return nc


def kernel(x, lora_matrix_B, lora_matrix_A):
    global _nc_cache, LAST_RESULTS
    from concourse.bass_utils import run_bass_kernel_spmd

    if _nc_cache is None:
        _nc_cache = _build_program()
    nc = _nc_cache

    x_flat = np.asarray(x, dtype=np.float32).reshape(NTOK, D).astype(np.float16)
    A = np.asarray(lora_matrix_A, dtype=np.float32).astype(np.float16)
    B = np.asarray(lora_matrix_B, dtype=np.float32).astype(np.float16)

    # at[p, ko, j] = A[j, ko*128 + p];  bt[j, o] = B[o, j]
    at_prep = np.ascontiguousarray(A.reshape(R, KO, P).transpose(2, 1, 0))
    bt_prep = np.ascontiguousarray(B.T)

    in_maps = []
    for c in range(NCORES):
        xc = x_flat[c * TOK : (c + 1) * TOK, :]
        # xt[p, ko, t] = xc[t, ko*128 + p]
        xt_prep = np.ascontiguousarray(xc.reshape(TOK, KO, P).transpose(2, 1, 0))
        in_maps.append({"xt": xt_prep, "at": at_prep, "bt": bt_prep})

    res = run_bass_kernel_spmd(
        nc, in_maps, core_ids=list(range(NCORES)), trace=TRACE
    )
    LAST_RESULTS = res

    y = np.concatenate(
        [np.asarray(res.results[c]["y"]) for c in range(NCORES)], axis=0
    )
    return y.reshape(BATCH, SEQ, D).astype(np.float32)
